# revision 23
# baseline (speedup 1.0000x reference)
"""Trainium2 Bass kernel for nn_AttentionLearnFusionDirectModule.

Takes FULL inputs, returns FULL output. Shards nseq=16 across 8 NeuronCores
(2 seqs/core, pure data parallel), runs one SPMD Bass program via
bass_utils.run_bass_kernel_spmd, gathers the output on host.

v2: host precomputes feature norms (tes pre-normalized, per-key exp scales
shipped as sclk), the bin-encode staging buffer is a [96, 384] multi-
partition layout (the old [8, 4608] layout cost ~7us per single-partition
DMA row), tanh is a Pade(7,6) rational on the Pool engine (keeps the ACT
table pinned on Exp), the conv tail runs in f32r with abs/clip fused into
one DVE bitwise op, and each sequence's tail work is software-pipelined
into the next sequences' sim loops.

Per-core pipeline (per sequence):
  1. cosine sim: f32r matmuls (c,k)x(c,q) per 128-wide k-tile; norms folded
     into the ACT Exp per-partition scale (host-computed)
  2. softmax-free attention: E = exp(temp*cos); num/den via one block-label
     matmul accumulated over all k-tiles; pmt = num * recip(den)
  3. bilinear 24->96 upsample as PE matmuls; mean/std over memories
  4. tanh (Pade on Pool), triangular soft-binning via a PE bin-center
     matmul, then f32r 1x1 convs (BN folded)
"""
import os
import sys

sys.path.insert(0, '/opt/trn_rl_repo')

import numpy as np
from contextlib import ExitStack

import concourse.bass as bass
import concourse.tile as tile
from concourse import mybir, bacc
from concourse.bass_utils import run_bass_kernel_spmd

f32 = mybir.dt.float32
f32r = mybir.dt.float32r
bf16 = mybir.dt.bfloat16
i32 = mybir.dt.int32
AF = mybir.ActivationFunctionType
ALU = mybir.AluOpType

M = 8
NSEQ = 16
C = 256
WF = HF = 24
WL = HL = 96
L = WF * HF            # 576
KTOT = M * L           # 4608
NCORES = 8
SL = NSEQ // NCORES    # 2
BN_EPS = 1e-5
NKT = KTOT // 128      # 36
PIX = WL * HL          # 9216
PIXH = PIX // 2        # 4608
NCH = 12               # tailB chunks per seq
CHW = PIXH // NCH      # 384 pixels per group per chunk
MAGIC = 0x5F3759DF
NEG1BITS = 0xBF800000  # float -1.0
SIGNBIT = 0x80000000
REPEAT = int(os.environ.get("KERNEL_REPEAT", "1"))
STAGE = int(os.environ.get("KERNEL_STAGE", "2"))  # 0: sims only, 1: +tailA, 2: full
NDLAG = int(os.environ.get("KERNEL_NDLAG", "1"))  # nd matmul lag in k-tiles


def resize_matrix(n_in, n_out):
    """Row-normalized triangle-kernel resize matrix matching jax.image.resize
    bilinear (antialias on downscale, half-pixel centers)."""
    scale = n_out / n_in
    kscale = min(scale, 1.0)
    R = np.zeros((n_out, n_in), np.float64)
    for i in range(n_out):
        c = (i + 0.5) / scale - 0.5
        w = np.maximum(0.0, 1.0 - np.abs(np.arange(n_in) - c) * kscale)
        R[i] = w / w.sum()
    return R.astype(np.float32)


def _rsqrt(nc, pool, x_ap, P, F, tag, n_iter=2):
    """1/sqrt(x) on DVE only (quake init + Newton). x: fp32 SBUF AP."""
    y = pool.tile([P, F], f32, tag=f"rsq_y{tag}")
    t1 = pool.tile([P, F], f32, tag=f"rsq_t{tag}")
    yi = y[:].bitcast(i32)
    nc.vector.tensor_scalar(yi, x_ap.bitcast(i32), 1, None,
                            op0=ALU.logical_shift_right)
    nc.vector.tensor_scalar(yi, yi, 0xFFFFFFFF, None, op0=ALU.bitwise_xor)
    nc.vector.tensor_scalar(yi, yi, MAGIC + 1, None, op0=ALU.add)
    for _ in range(n_iter):
        nc.vector.tensor_tensor(t1[:], y[:], y[:], op=ALU.mult)
        nc.vector.tensor_tensor(t1[:], t1[:], x_ap, op=ALU.mult)
        nc.vector.tensor_scalar(t1[:], t1[:], -0.5, 1.5, op0=ALU.mult, op1=ALU.add)
        nc.vector.tensor_tensor(y[:], y[:], t1[:], op=ALU.mult)
    return y


def build_program():
    nc = bacc.Bacc("TRN2", target_bir_lowering=False, debug=False,
                   num_devices=NCORES)

    trf = nc.dram_tensor("trf", [SL, 2, 128, KTOT], f32r, kind="ExternalInput").ap()
    tesd = nc.dram_tensor("tesd", [SL, 2, 128, L], f32r, kind="ExternalInput").ap()
    sclki = nc.dram_tensor("sclki", [SL, 128, NKT], f32, kind="ExternalInput").ap()
    lmat = nc.dram_tensor("lmat", [SL, 128, NKT * 40], f32r, kind="ExternalInput").ap()
    tsc = nc.dram_tensor("tsc", [SL, 96, 96], f32, kind="ExternalInput").ap()
    w1 = nc.dram_tensor("w1", [128, 128], f32r, kind="ExternalInput").ap()
    w2 = nc.dram_tensor("w2", [128, 2], f32r, kind="ExternalInput").ap()
    b1 = nc.dram_tensor("b1", [128, 1], f32, kind="ExternalInput").ap()
    binlhs = nc.dram_tensor("binlhs", [12, 96, 128], f32r, kind="ExternalInput").ap()
    utm = nc.dram_tensor("utm", [24, 96], f32r, kind="ExternalInput").ap()
    brini = nc.dram_tensor("brini", [96, CHW], f32r, kind="ExternalInput").ap()
    outd = nc.dram_tensor("out", [SL, 2, PIXH], f32, kind="ExternalOutput").ap()
    # dram scratch for layout shuffles
    pmt_scr = nc.dram_tensor("pmt_scr", [SL, KTOT], f32r, kind="Internal").ap()
    th_scr = nc.dram_tensor("th_scr", [SL, 3, PIX], f32r, kind="Internal").ap()

    with tile.TileContext(nc) as tc, ExitStack() as ctx:
        big = ctx.enter_context(tc.tile_pool(name="big", bufs=2))
        sml = ctx.enter_context(tc.tile_pool(name="sml", bufs=2))
        epool = ctx.enter_context(tc.tile_pool(name="epool", bufs=4))
        cst = ctx.enter_context(tc.tile_pool(name="cst", bufs=1))
        tails = ctx.enter_context(tc.tile_pool(name="tails", bufs=2))
        lmp = ctx.enter_context(tc.tile_pool(name="lmp", bufs=2))
        chks = ctx.enter_context(tc.tile_pool(name="chks", bufs=2))
        c2p = ctx.enter_context(tc.tile_pool(name="c2p", bufs=2))
        simp = ctx.enter_context(tc.tile_pool(name="simp", bufs=2, space="PSUM"))
        ndp = ctx.enter_context(tc.tile_pool(name="ndp", bufs=1, space="PSUM"))
        tbp = ctx.enter_context(tc.tile_pool(name="tbp", bufs=2, space="PSUM"))

        # persistent constants
        w1sb = cst.tile([128, 128], f32r); nc.sync.dma_start(w1sb[:], w1[:, :])
        w2sb = cst.tile([128, 2], f32r);   nc.sync.dma_start(w2sb[:], w2[:, :])
        b1sb = cst.tile([128, 1], f32);    nc.sync.dma_start(b1sb[:], b1[:, :])
        blsb = []
        for bb in range(12):
            t = cst.tile([96, 128], f32r, name=f"blsb{bb}")
            nc.sync.dma_start(t[:], binlhs[bb])
            blsb.append(t)
        utsb = cst.tile([24, 96], f32r);   nc.sync.dma_start(utsb[:], utm[:, :])
        zsb = cst.tile([128, 1], f32);     nc.vector.memset(zsb[:], 0.0)
        brhsP = []
        for bb in range(2):
            t = cst.tile([96, CHW], f32r, name=f"brhs{bb}")
            nc.sync.dma_start(t[:], brini[:, :])
            brhsP.append(t)

        def load(s):
            """Emit input DMAs for sequence slot s; returns tile handles."""
            d = {"s": s}
            te0_t = sml.tile([128, L], f32r, tag="te0")
            nc.sync.dma_start(te0_t[:], tesd[s, 0])
            te1_t = sml.tile([128, L], f32r, tag="te1")
            nc.sync.dma_start(te1_t[:], tesd[s, 1])
            sclk_t = sml.tile([128, NKT], f32, tag="sclk")
            nc.sync.dma_start(sclk_t[:], sclki[s])
            scsb_t = sml.tile([96, 96], f32, tag="scsb")
            nc.sync.dma_start(scsb_t[:], tsc[s])
            lmsb_t = lmp.tile([128, NKT * 40], f32r, tag="lmsb")
            nc.sync.dma_start(lmsb_t[:], lmat[s])
            d.update(te0=te0_t, te1=te1_t, sclk=sclk_t, scsb=scsb_t, lmsb=lmsb_t)
            tr0_t = big.tile([128, KTOT], f32r, tag="tr0")
            tr1_t = big.tile([128, KTOT], f32r, tag="tr1")
            d.update(tr0=tr0_t, tr1=tr1_t)
            for q0 in range(0, KTOT, 2304):
                nc.sync.dma_start(tr0_t[:, q0:q0 + 2304], trf[s, 0, :, q0:q0 + 2304])
                nc.sync.dma_start(tr1_t[:, q0:q0 + 2304], trf[s, 1, :, q0:q0 + 2304])
            return d

        def sim(d, weave=None):
            tr0, tr1, te0, te1 = d["tr0"], d["tr1"], d["te0"], d["te1"]
            lmsb, sclk = d["lmsb"], d["sclk"]
            ndps = ndp.tile([40, 2, 512], f32, tag="ndps")
            d["ndps"] = ndps

            def nd_mm(t, et):
                for n in range(2):
                    nc.tensor.matmul(ndps[:, n, 0:288],
                                     lmsb[:, t * 40:(t + 1) * 40],
                                     et[:, n, 0:288],
                                     start=(t == 0), stop=(t == NKT - 1))

            ets = {}
            for t in range(NKT):
                sps = simp.tile([128, 2, 512], f32, tag="sps")
                for cc, (tr, te) in enumerate(((tr0, te0), (tr1, te1))):
                    for n, no in enumerate((0, 288)):
                        nc.tensor.matmul(sps[:, n, 0:288],
                                         tr[:, t * 128:(t + 1) * 128],
                                         te[:, no:no + 288],
                                         start=(cc == 0), stop=(cc == 1))
                et = epool.tile([128, 2, 288], f32r, tag="et")
                nc.scalar.activation(et[:], sps[:, 0:2, 0:288], AF.Exp,
                                     bias=zsb[:, 0:1], scale=sclk[:, t:t + 1])
                ets[t] = et
                if t - NDLAG >= 0:
                    nd_mm(t - NDLAG, ets.pop(t - NDLAG))
                if weave is not None:
                    weave(t)
            for t in sorted(ets):
                nd_mm(t, ets.pop(t))

        def pmtevac(d, st):
            s = d["s"]
            ndps = d["ndps"]
            recd = tails.tile([8, 2, 288], f32, tag="recd")
            nc.vector.reciprocal(recd[:], ndps[32:40, 0:2, 0:288])
            pmt = tails.tile([8, 2, 288], f32r, tag="pmt")
            nc.vector.tensor_tensor(pmt[:], ndps[0:8, 0:2, 0:288], recd[:], op=ALU.mult)
            nc.sync.dma_start(pmt_scr[s].rearrange("(m q) -> m q", m=8),
                                pmt[:].rearrange("m n w -> m (n w)"))
            p24 = tails.tile([24, 192], f32r, tag="p24")
            nc.sync.dma_start(
                p24[:].rearrange("i (m j) -> i m j", m=8),
                pmt_scr[s].rearrange("(m i j) -> i m j", m=8, i=24))
            st["p24"] = p24

        # ---- tailA pieces: upsample + mean/std + tanh + staging ----

        def pA1(d, st):
            p24 = st["p24"]
            t1t = tails.tile([24, 768], f32r, tag="t1t")
            st["t1t"] = t1t
            for half in range(2):
                t1p = tbp.tile([24, 512], f32, tag="tb")
                for mm in range(4):
                    m = half * 4 + mm
                    nc.tensor.matmul(t1p[:, mm * 96:(mm + 1) * 96],
                                     p24[:, m * 24:(m + 1) * 24], utsb[:],
                                     start=True, stop=True)
                nc.vector.tensor_copy(t1t[:, half * 384:(half + 1) * 384],
                                      t1p[:, 0:384])

        def pA2(d, st):
            t1t = st["t1t"]
            uS = tails.tile([96, 768], f32, tag="uS")
            uQ = tails.tile([96, 768], f32, tag="uQ")
            st["uS"], st["uQ"] = uS, uQ
            for o, w in ((0, 512), (512, 256)):
                ups = tbp.tile([96, 512], f32, tag="tb")
                nc.tensor.matmul(ups[:, 0:w], utsb[:], t1t[:, o:o + w],
                                 start=True, stop=True)
                nc.vector.tensor_copy(uS[:, o:o + w], ups[:, 0:w])
                nc.scalar.activation(uQ[:, o:o + w], ups[:, 0:w], AF.Square,
                                     bias=zsb[0:96, 0:1])

        def pA3(d, st):
            uS, uQ = st["uS"], st["uQ"]
            sU = tails.tile([96, 96], f32, tag="sU")
            nc.vector.tensor_reduce(sU[:], uS[:].rearrange("x (m y) -> x y m", m=8),
                                    axis=mybir.AxisListType.X, op=ALU.add)
            sQ = tails.tile([96, 96], f32, tag="sQ")
            nc.vector.tensor_reduce(sQ[:], uQ[:].rearrange("x (m y) -> x y m", m=8),
                                    axis=mybir.AxisListType.X, op=ALU.add)
            m2 = tails.tile([96, 96], f32, tag="m2")
            nc.vector.tensor_tensor(m2[:], sU[:], sU[:], op=ALU.mult)
            nc.vector.tensor_scalar(m2[:], m2[:], -0.125, None, op0=ALU.mult)
            nc.vector.tensor_tensor(m2[:], m2[:], sQ[:], op=ALU.add)
            nc.vector.tensor_scalar(m2[:], m2[:], 1.0 / 7.0, 1e-30,
                                    op0=ALU.mult, op1=ALU.max)
            rv = _rsqrt(nc, tails, m2[:], 96, 96, tag="v", n_iter=1)
            st["sU"], st["m2"], st["rv"] = sU, m2, rv

        def pA4(d, st):
            # X = [tanh-in maps: scores | mean | std] then Pade(7,6) on Pool
            scsb = d["scsb"]
            sU, m2, rv = st["sU"], st["m2"], st["rv"]
            X = tails.tile([96, 288], f32, tag="X")
            nc.vector.tensor_copy(X[:, 0:96], scsb[:])
            nc.vector.tensor_scalar(X[:, 96:192], sU[:], 0.125, None, op0=ALU.mult)
            nc.vector.tensor_tensor(X[:, 192:288], m2[:], rv[:], op=ALU.mult)
            st["X"] = X

        def pA5(d, st):
            X = st["X"]
            t = tails.tile([96, 288], f32, tag="th_t")
            n = tails.tile([96, 288], f32, tag="th_n")
            nc.vector.tensor_tensor(t[:], X[:], X[:], op=ALU.mult)
            nc.vector.tensor_scalar(n[:], t[:], 378.0, None, op0=ALU.add)
            nc.vector.tensor_tensor(n[:], n[:], t[:], op=ALU.mult)
            nc.vector.tensor_scalar(n[:], n[:], 17325.0, None, op0=ALU.add)
            nc.vector.tensor_tensor(n[:], n[:], t[:], op=ALU.mult)
            nc.vector.tensor_scalar(n[:], n[:], 135135.0, None, op0=ALU.add)
            nc.vector.tensor_tensor(n[:], n[:], X[:], op=ALU.mult)
            st["th_t"], st["th_n"] = t, n

        def pA6(d, st):
            t, n = st["th_t"], st["th_n"]
            dn = tails.tile([96, 288], f32, tag="th_d")
            nc.vector.tensor_scalar(dn[:], t[:], 28.0, 3150.0, op0=ALU.mult, op1=ALU.add)
            nc.vector.tensor_tensor(dn[:], dn[:], t[:], op=ALU.mult)
            nc.vector.tensor_scalar(dn[:], dn[:], 62370.0, None, op0=ALU.add)
            nc.vector.tensor_tensor(dn[:], dn[:], t[:], op=ALU.mult)
            nc.vector.tensor_scalar(dn[:], dn[:], 135135.0, None, op0=ALU.add)
            nc.vector.reciprocal(dn[:], dn[:])
            thv = tails.tile([96, 288], f32r, tag="thv")
            nc.vector.tensor_tensor(thv[:], n[:], dn[:], op=ALU.mult)
            nc.vector.tensor_scalar(thv[:], thv[:], -1.0, 1.0, op0=ALU.max, op1=ALU.min)
            st["thv"] = thv

        def pF(d, st):
            s = d["s"]
            thv = st["thv"]
            brhs = brhsP[st["slot"] % 2]
            st["brhs"] = brhs
            for j in range(3):
                nc.sync.dma_start(
                    th_scr[s, j].rearrange("(a b) -> a b", a=96),
                    thv[:, j * 96:(j + 1) * 96])
            for j in range(3):
                for g in range(2):
                    nc.sync.dma_start(
                        brhs[:].rearrange("(sr gj) q -> gj sr q", gj=8)[4 * g + j],
                        th_scr[s, j, g * PIXH:(g + 1) * PIXH]
                        .rearrange("(sr q) -> sr q", q=CHW))
            c2sb = c2p.tile([128, 72], f32, tag="c2sb")
            st["c2sb"] = c2sb
            st["enc"] = {}
            st["r1"] = {}
            st["c2src"] = {}

        def chunk(d, st, k):
            brhs, c2sb = st["brhs"], st["c2sb"]
            if k < NCH:
                D = tbp.tile([128, 512], f32, tag="tb")
                nc.tensor.matmul(D[:, 0:CHW], blsb[k][:], brhs[:],
                                 start=True, stop=True)
                e1 = chks.tile([128, CHW], f32, tag="e1")
                nc.scalar.activation(e1[:], D[:, 0:CHW], AF.Abs, bias=zsb[:, 0:1])
                enc = chks.tile([128, CHW], f32r, tag="enc")
                nc.vector.tensor_scalar(enc[:], e1[:], -1.0, -1.0,
                                        op0=ALU.mult, op1=ALU.max)
                st["enc"][k] = enc
            if 0 <= k - 1 < NCH or 0 <= k - 2 < NCH:
                c1 = tbp.tile([128, 512], f32, tag="tb")
                kk = k - 2
                if 0 <= kk < NCH:
                    # conv2 for chunk k-2 rides in spare cols of this psum tile
                    for nn in range(3):
                        nc.tensor.matmul(c1[:, CHW + 2 * nn:CHW + 2 * nn + 2],
                                         st["r1"][kk][:, nn * 128:(nn + 1) * 128],
                                         w2sb[:], start=True, stop=True)
                    st["c2src"][kk] = c1
                if 0 <= k - 1 < NCH:
                    nc.tensor.matmul(c1[:, 0:CHW], w1sb[:], st["enc"].pop(k - 1)[:],
                                     start=True, stop=True)
                    r1 = chks.tile([128, CHW], f32r, tag="r1")
                    nc.vector.tensor_scalar(r1[:], c1[:, 0:CHW], b1sb[:, 0:1], 0.0,
                                            op0=ALU.add, op1=ALU.max)
                    st["r1"][k - 1] = r1
            kk = k - 2
            if 0 <= kk < NCH and kk in st["c2src"]:
                src = st["c2src"].pop(kk)
                st["r1"].pop(kk)
                nc.vector.tensor_copy(
                    c2sb[:].rearrange("p (g cc) -> p g cc", g=2)[:, :, 3 * kk:3 * kk + 3],
                    src[:, CHW:CHW + 6].rearrange("p (n g) -> p g n", g=2))

        def fin(d, st):
            s = d["s"]
            nc.sync.dma_start(
                outd[s].rearrange("g (c p) -> p g c", p=128),
                st["c2sb"][:].rearrange("p (g c) -> p g c", g=2))

        def tail_pieces(d, st):
            pa = [lambda: pA1(d, st), lambda: pA2(d, st), lambda: pA3(d, st),
                  lambda: pA4(d, st), lambda: pA5(d, st), lambda: pA6(d, st),
                  lambda: pF(d, st)]
            pb = [(lambda k: (lambda: chunk(d, st, k)))(k) for k in range(NCH + 2)]
            pb.append(lambda: fin(d, st))
            return pa, pb

        QA_SLOTS = {12, 15, 18, 21, 24, 27, 30}

        def weaver(qa, qb):
            def w(t):
                if t in QA_SLOTS and qa:
                    qa.pop(0)()
                elif qb and ((t % 2 == 1 and t >= 5 and t not in QA_SLOTS)
                             or t >= 31):
                    qb.pop(0)()
            return w

        # ---- main pipelined flow ----
        # tailA of seq i weaves into sim(i+1); its conv chunks into sim(i+2)
        NS = 2 * REPEAT
        qa, qb, qb_next = [], [], []
        H = {0: load(0)}
        for i in range(NS):
            if i + 1 < NS:
                H[i + 1] = load((i + 1) % SL)
            d = H.pop(i)
            sim(d, weave=weaver(qa, qb))
            for p in qa + qb:
                p()
            if STAGE >= 1:
                st = {"slot": i}
                pmtevac(d, st)
                qa, newb = tail_pieces(d, st)
                qb = qb_next
                qb_next = newb if STAGE >= 2 else []
            else:
                qa, qb, qb_next = [], [], []
        for p in qa + qb + qb_next:
            p()

    nc.compile()
    return nc


_prog = None


def kernel(**inputs) -> np.ndarray:
    global _prog
    test_scores = np.asarray(inputs["test_scores"], np.float32)
    train_labels = np.asarray(inputs["train_labels"], np.float32)
    test_feat = np.asarray(inputs["test_feat"], np.float32)
    train_feats = np.asarray(inputs["train_feats"], np.float32)
    temp = float(np.asarray(inputs["softmax_temp"]).reshape(-1)[0])
    conv1_w = np.asarray(inputs["conv1_w"], np.float32)
    conv1_b = np.asarray(inputs["conv1_b"], np.float32)
    bn_gamma = np.asarray(inputs["bn_gamma"], np.float32)
    bn_beta = np.asarray(inputs["bn_beta"], np.float32)
    bn_mean = np.asarray(inputs["bn_mean"], np.float32)
    bn_var = np.asarray(inputs["bn_var"], np.float32)
    conv2_w = np.asarray(inputs["conv2_w"], np.float32)
    conv2_b = np.asarray(inputs["conv2_b"], np.float32)

    R = resize_matrix(96, 24)
    labd = np.einsum("ik,mskl,jl->msij", R, train_labels, R)  # (M, NSEQ, 24, 24)
    lm_all = np.zeros((NSEQ, KTOT, 40), np.float32)
    for m in range(M):
        lm_all[:, m * L:(m + 1) * L, m] = labd[m].reshape(NSEQ, L)
        lm_all[:, m * L:(m + 1) * L, 32 + m] = 1.0
    lm_dev = lm_all.reshape(NSEQ, NKT, 128, 40).transpose(0, 2, 1, 3) \
        .reshape(NSEQ, 128, NKT * 40)

    s_o = np.sqrt(bn_var + BN_EPS)
    w1f = conv1_w * (bn_gamma / s_o)[:, None]
    b1f = (conv1_b - bn_mean) / s_o * bn_gamma + bn_beta
    b1f = b1f + w1f.sum(axis=1)   # kernel feeds enc-1; fold +1*W1 into bias
    W1 = np.zeros((128, 128), np.float32)
    W1[0:64, 0:64] = w1f.T
    W1[64:128, 64:128] = w1f.T
    W2 = np.zeros((128, 2), np.float32)
    W2[0:64, 0] = conv2_w[0]
    W2[64:128, 1] = conv2_w[0]
    B1 = np.concatenate([b1f, b1f]).reshape(128, 1)

    BL = np.zeros((8, 128), np.float32)
    for g in range(2):
        for ch in range(64):
            p = ch + 64 * g
            if ch < 32:
                j, a, b, c = 0, 15.5, 15.5, float(ch)
            elif ch < 48:
                j, a, b, c = 1, 15.0, 0.0, float(ch - 32)
            else:
                j, a, b, c = 2, 15.0, 0.0, float(ch - 48)
            BL[4 * g + j, p] = a
            BL[4 * g + 3, p] += b - c
    BL12 = np.zeros((12, 96, 128), np.float32)
    for bb in range(12):
        BL12[bb, 8 * bb:8 * bb + 8] = BL
    BRI = np.zeros((96, CHW), np.float32)
    for sr in range(NCH):
        for g in range(2):
            BRI[sr * 8 + 4 * g + 3, :] = 1.0

    UT = np.ascontiguousarray(resize_matrix(24, 96).T)

    tf_r = train_feats.reshape(M, NSEQ, C, L)
    te_r = test_feat.reshape(NSEQ, C, L)
    # host-computed norms: tes pre-normalized, per-key exp scale = temp/||tr_k||
    tes_all = te_r / np.sqrt((te_r * te_r).sum(axis=1, keepdims=True))
    nk = np.sqrt((tf_r * tf_r).sum(axis=2))          # (M, NSEQ, L)
    nkk = nk.transpose(1, 0, 2).reshape(NSEQ, KTOT)  # key order k = m*L + l
    sclk_all = (temp / nkk).reshape(NSEQ, NKT, 128).transpose(0, 2, 1)

    in_maps = []
    for c in range(NCORES):
        sl = slice(SL * c, SL * (c + 1))
        trc = np.ascontiguousarray(
            tf_r[:, sl].transpose(1, 2, 0, 3).reshape(SL, 2, 128, KTOT))
        tec = np.ascontiguousarray(tes_all[sl].reshape(SL, 2, 128, L))
        tscc = np.ascontiguousarray(np.transpose(test_scores[0, sl], (0, 2, 1)))
        in_maps.append({
            "trf": trc, "tesd": tec,
            "sclki": np.ascontiguousarray(sclk_all[sl]),
            "lmat": np.ascontiguousarray(lm_dev[sl]),
            "tsc": tscc,
            "w1": W1, "w2": W2, "b1": B1, "binlhs": BL12, "utm": UT,
            "brini": BRI,
        })

    if _prog is None:
        _prog = build_program()
    res = run_bass_kernel_spmd(_prog, in_maps, core_ids=list(range(NCORES)))

    out = np.empty((1, NSEQ, WL, HL), np.float32)
    for c in range(NCORES):
        o = res.results[c]["out"]
        for s in range(SL):
            img_t = np.concatenate([o[s, 0], o[s, 1]]).reshape(96, 96)
            out[0, SL * c + s] = img_t.T + conv2_b[0]
    return out


if __name__ == "__main__":
    rng = np.random.default_rng(0)
    ins = {
        "test_scores": rng.standard_normal((1, NSEQ, WL, HL)).astype(np.float32),
        "train_labels": rng.uniform(0, 1, (M, NSEQ, WL, HL)).astype(np.float32),
        "test_feat": rng.standard_normal((1, NSEQ, C, WF, HF)).astype(np.float32),
        "train_feats": rng.standard_normal((M, NSEQ, C, WF, HF)).astype(np.float32),
        "softmax_temp": np.full((1,), 50.0, np.float32),
        "conv1_w": (rng.standard_normal((64, 64)) * 0.05).astype(np.float32),
        "conv1_b": np.zeros((64,), np.float32),
        "bn_gamma": np.ones((64,), np.float32),
        "bn_beta": np.zeros((64,), np.float32),
        "bn_mean": np.zeros((64,), np.float32),
        "bn_var": np.ones((64,), np.float32),
        "conv2_w": (rng.standard_normal((1, 64)) * 0.05).astype(np.float32),
        "conv2_b": np.zeros((1,), np.float32),
    }
    out = kernel(**ins)
    print("out shape:", out.shape, "mean", float(out.mean()), "std", float(out.std()))


# revision 25
# speedup vs baseline: 1.1898x; 1.1898x over previous
"""Trainium2 Bass kernel for nn_AttentionLearnFusionDirectModule.

Takes FULL inputs, returns FULL output. Shards nseq=16 across 8 NeuronCores
(2 seqs/core, pure data parallel), runs one SPMD Bass program via
bass_utils.run_bass_kernel_spmd, gathers the output on host.

v2: host precomputes feature norms (tes pre-normalized, per-key exp scales
shipped as sclk), the bin-encode staging buffer is a [96, 384] multi-
partition layout (the old [8, 4608] layout cost ~7us per single-partition
DMA row), tanh is a Pade(7,6) rational on the Pool engine (keeps the ACT
table pinned on Exp), the conv tail runs in f32r with abs/clip fused into
one DVE bitwise op, and each sequence's tail work is software-pipelined
into the next sequences' sim loops.

Per-core pipeline (per sequence):
  1. cosine sim: f32r matmuls (c,k)x(c,q) per 128-wide k-tile; norms folded
     into the ACT Exp per-partition scale (host-computed)
  2. softmax-free attention: E = exp(temp*cos); num/den via one block-label
     matmul accumulated over all k-tiles; pmt = num * recip(den)
  3. bilinear 24->96 upsample as PE matmuls; mean/std over memories
  4. tanh (Pade on Pool), triangular soft-binning via a PE bin-center
     matmul, then f32r 1x1 convs (BN folded)
"""
import os
import sys

sys.path.insert(0, '/opt/trn_rl_repo')

import numpy as np
from contextlib import ExitStack

import concourse.bass as bass
import concourse.tile as tile
from concourse import mybir, bacc
from concourse.bass_utils import run_bass_kernel_spmd

f32 = mybir.dt.float32
f32r = mybir.dt.float32r
bf16 = mybir.dt.bfloat16
i32 = mybir.dt.int32
AF = mybir.ActivationFunctionType
ALU = mybir.AluOpType

M = 8
NSEQ = 16
C = 256
WF = HF = 24
WL = HL = 96
L = WF * HF            # 576
KTOT = M * L           # 4608
NCORES = 8
SL = NSEQ // NCORES    # 2
BN_EPS = 1e-5
NKT = KTOT // 128      # 36
PIX = WL * HL          # 9216
PIXH = PIX // 2        # 4608
NCH = 12               # tailB chunks per seq
CHW = PIXH // NCH      # 384 pixels per group per chunk
MAGIC = 0x5F3759DF
NEG1BITS = 0xBF800000  # float -1.0
SIGNBIT = 0x80000000
REPEAT = int(os.environ.get("KERNEL_REPEAT", "1"))
STAGE = int(os.environ.get("KERNEL_STAGE", "2"))  # 0: sims only, 1: +tailA, 2: full
NDLAG = int(os.environ.get("KERNEL_NDLAG", "2"))  # nd matmul lag in k-tiles


def resize_matrix(n_in, n_out):
    """Row-normalized triangle-kernel resize matrix matching jax.image.resize
    bilinear (antialias on downscale, half-pixel centers)."""
    scale = n_out / n_in
    kscale = min(scale, 1.0)
    R = np.zeros((n_out, n_in), np.float64)
    for i in range(n_out):
        c = (i + 0.5) / scale - 0.5
        w = np.maximum(0.0, 1.0 - np.abs(np.arange(n_in) - c) * kscale)
        R[i] = w / w.sum()
    return R.astype(np.float32)


def _rsqrt(nc, pool, x_ap, P, F, tag, n_iter=2):
    """1/sqrt(x) on DVE only (quake init + Newton). x: fp32 SBUF AP."""
    y = pool.tile([P, F], f32, tag=f"rsq_y{tag}")
    t1 = pool.tile([P, F], f32, tag=f"rsq_t{tag}")
    yi = y[:].bitcast(i32)
    nc.vector.tensor_scalar(yi, x_ap.bitcast(i32), 1, None,
                            op0=ALU.logical_shift_right)
    nc.vector.tensor_scalar(yi, yi, 0xFFFFFFFF, None, op0=ALU.bitwise_xor)
    nc.vector.tensor_scalar(yi, yi, MAGIC + 1, None, op0=ALU.add)
    for _ in range(n_iter):
        nc.vector.tensor_tensor(t1[:], y[:], y[:], op=ALU.mult)
        nc.vector.tensor_tensor(t1[:], t1[:], x_ap, op=ALU.mult)
        nc.vector.tensor_scalar(t1[:], t1[:], -0.5, 1.5, op0=ALU.mult, op1=ALU.add)
        nc.vector.tensor_tensor(y[:], y[:], t1[:], op=ALU.mult)
    return y


def build_program():
    nc = bacc.Bacc("TRN2", target_bir_lowering=False, debug=False,
                   num_devices=NCORES)

    trf = nc.dram_tensor("trf", [SL, 2, 128, KTOT], f32r, kind="ExternalInput").ap()
    PKW = 2 * L + NKT + NKT * 40   # tes0|tes1|sclk|lmat packed columns
    pkin = nc.dram_tensor("pkin", [SL, 128, PKW], f32r, kind="ExternalInput").ap()
    tsc = nc.dram_tensor("tsc", [SL, 96, 96], f32, kind="ExternalInput").ap()
    w1 = nc.dram_tensor("w1", [128, 128], f32r, kind="ExternalInput").ap()
    w2 = nc.dram_tensor("w2", [128, 2], f32r, kind="ExternalInput").ap()
    b1 = nc.dram_tensor("b1", [128, 1], f32, kind="ExternalInput").ap()
    binlhs = nc.dram_tensor("binlhs", [12, 96, 128], f32r, kind="ExternalInput").ap()
    utm = nc.dram_tensor("utm", [24, 96], f32r, kind="ExternalInput").ap()
    brini = nc.dram_tensor("brini", [96, CHW], f32r, kind="ExternalInput").ap()
    outd = nc.dram_tensor("out", [SL, 2, PIXH], f32, kind="ExternalOutput").ap()
    # dram scratch for layout shuffles
    pmt_scr = nc.dram_tensor("pmt_scr", [SL, KTOT], f32r, kind="Internal").ap()
    th_scr = nc.dram_tensor("th_scr", [SL, 3, PIX], f32r, kind="Internal").ap()

    with tile.TileContext(nc) as tc, ExitStack() as ctx:
        big = ctx.enter_context(tc.tile_pool(name="big", bufs=2))
        sml = ctx.enter_context(tc.tile_pool(name="sml", bufs=2))
        epool = ctx.enter_context(tc.tile_pool(name="epool", bufs=4))
        cst = ctx.enter_context(tc.tile_pool(name="cst", bufs=1))
        tails = ctx.enter_context(tc.tile_pool(name="tails", bufs=2))
        lmp = ctx.enter_context(tc.tile_pool(name="lmp", bufs=2))
        chks = ctx.enter_context(tc.tile_pool(name="chks", bufs=2))
        c2p = ctx.enter_context(tc.tile_pool(name="c2p", bufs=2))
        simp = ctx.enter_context(tc.tile_pool(name="simp", bufs=2, space="PSUM"))
        ndp = ctx.enter_context(tc.tile_pool(name="ndp", bufs=1, space="PSUM"))
        tbp = ctx.enter_context(tc.tile_pool(name="tbp", bufs=2, space="PSUM"))

        # persistent constants
        w1sb = cst.tile([128, 128], f32r); nc.sync.dma_start(w1sb[:], w1[:, :])
        w2sb = cst.tile([128, 2], f32r);   nc.sync.dma_start(w2sb[:], w2[:, :])
        b1sb = cst.tile([128, 1], f32);    nc.sync.dma_start(b1sb[:], b1[:, :])
        blsb = []
        for bb in range(12):
            t = cst.tile([96, 128], f32r, name=f"blsb{bb}")
            nc.sync.dma_start(t[:], binlhs[bb])
            blsb.append(t)
        utsb = cst.tile([24, 96], f32r);   nc.sync.dma_start(utsb[:], utm[:, :])
        zsb = cst.tile([128, 1], f32);     nc.vector.memset(zsb[:], 0.0)
        brhsP = []
        for bb in range(2):
            t = cst.tile([96, CHW], f32r, name=f"brhs{bb}")
            nc.sync.dma_start(t[:], brini[:, :])
            brhsP.append(t)

        def load(s):
            """Emit input DMAs for sequence slot s; returns tile handles."""
            d = {"s": s}
            pk_t = lmp.tile([128, PKW], f32r, tag="pk")
            nc.sync.dma_start(pk_t[:], pkin[s])
            scsb_t = sml.tile([96, 96], f32, tag="scsb")
            nc.sync.dma_start(scsb_t[:], tsc[s])
            d.update(te0=pk_t[:, 0:L], te1=pk_t[:, L:2 * L],
                     sclk=pk_t[:, 2 * L:2 * L + NKT].bitcast(f32),
                     scsb=scsb_t, lmsb=pk_t[:, 2 * L + NKT:PKW])
            tr0_t = big.tile([128, KTOT], f32r, tag="tr0")
            tr1_t = big.tile([128, KTOT], f32r, tag="tr1")
            d.update(tr0=tr0_t, tr1=tr1_t)
            for q0 in range(0, KTOT, 2304):
                nc.sync.dma_start(tr0_t[:, q0:q0 + 2304], trf[s, 0, :, q0:q0 + 2304])
                nc.sync.dma_start(tr1_t[:, q0:q0 + 2304], trf[s, 1, :, q0:q0 + 2304])
            return d

        def sim(d, weave=None):
            tr0, tr1, te0, te1 = d["tr0"], d["tr1"], d["te0"], d["te1"]
            lmsb, sclk = d["lmsb"], d["sclk"]
            ndps = ndp.tile([40, 2, 512], f32, tag="ndps")
            d["ndps"] = ndps

            def nd_mm(t, et):
                for n in range(2):
                    nc.tensor.matmul(ndps[:, n, 0:288],
                                     lmsb[:, t * 40:(t + 1) * 40],
                                     et[:, n, 0:288],
                                     start=(t == 0), stop=(t == NKT - 1))

            ets = {}
            for t in range(NKT):
                sps = simp.tile([128, 2, 512], f32, tag="sps")
                for cc, (tr, te) in enumerate(((tr0, te0), (tr1, te1))):
                    for n, no in enumerate((0, 288)):
                        nc.tensor.matmul(sps[:, n, 0:288],
                                         tr[:, t * 128:(t + 1) * 128],
                                         te[:, no:no + 288],
                                         start=(cc == 0), stop=(cc == 1))
                et = epool.tile([128, 2, 288], f32r, tag="et")
                nc.scalar.activation(et[:], sps[:, 0:2, 0:288], AF.Exp,
                                     bias=zsb[:, 0:1], scale=sclk[:, t:t + 1])
                ets[t] = et
                if t - NDLAG >= 0:
                    nd_mm(t - NDLAG, ets.pop(t - NDLAG))
                if weave is not None:
                    weave(t)
            for t in sorted(ets):
                nd_mm(t, ets.pop(t))

        def pmtevac(d, st):
            s = d["s"]
            ndps = d["ndps"]
            recd = tails.tile([8, 2, 288], f32, tag="recd")
            nc.vector.reciprocal(recd[:], ndps[32:40, 0:2, 0:288])
            pmt = tails.tile([8, 2, 288], f32r, tag="pmt")
            nc.vector.tensor_tensor(pmt[:], ndps[0:8, 0:2, 0:288], recd[:], op=ALU.mult)
            nc.sync.dma_start(pmt_scr[s].rearrange("(m q) -> m q", m=8),
                                pmt[:].rearrange("m n w -> m (n w)"))
            p24 = tails.tile([24, 192], f32r, tag="p24")
            nc.sync.dma_start(
                p24[:].rearrange("i (m j) -> i m j", m=8),
                pmt_scr[s].rearrange("(m i j) -> i m j", m=8, i=24))
            st["p24"] = p24

        # ---- tailA pieces: upsample + mean/std + tanh + staging ----

        def pA1(d, st):
            p24 = st["p24"]
            t1t = tails.tile([24, 768], f32r, tag="t1t")
            st["t1t"] = t1t
            for half in range(2):
                t1p = tbp.tile([24, 512], f32, tag="tb")
                for mm in range(4):
                    m = half * 4 + mm
                    nc.tensor.matmul(t1p[:, mm * 96:(mm + 1) * 96],
                                     p24[:, m * 24:(m + 1) * 24], utsb[:],
                                     start=True, stop=True)
                nc.vector.tensor_copy(t1t[:, half * 384:(half + 1) * 384],
                                      t1p[:, 0:384])

        def pA2(d, st):
            t1t = st["t1t"]
            uS = tails.tile([96, 768], f32, tag="uS")
            uQ = tails.tile([96, 768], f32, tag="uQ")
            st["uS"], st["uQ"] = uS, uQ
            for o, w in ((0, 512), (512, 256)):
                ups = tbp.tile([96, 512], f32, tag="tb")
                nc.tensor.matmul(ups[:, 0:w], utsb[:], t1t[:, o:o + w],
                                 start=True, stop=True)
                nc.vector.tensor_copy(uS[:, o:o + w], ups[:, 0:w])
                nc.scalar.activation(uQ[:, o:o + w], ups[:, 0:w], AF.Square,
                                     bias=zsb[0:96, 0:1])

        def pA3(d, st):
            uS, uQ = st["uS"], st["uQ"]
            sU = tails.tile([96, 96], f32, tag="sU")
            nc.vector.tensor_reduce(sU[:], uS[:].rearrange("x (m y) -> x y m", m=8),
                                    axis=mybir.AxisListType.X, op=ALU.add)
            sQ = tails.tile([96, 96], f32, tag="sQ")
            nc.vector.tensor_reduce(sQ[:], uQ[:].rearrange("x (m y) -> x y m", m=8),
                                    axis=mybir.AxisListType.X, op=ALU.add)
            m2 = tails.tile([96, 96], f32, tag="m2")
            nc.vector.tensor_tensor(m2[:], sU[:], sU[:], op=ALU.mult)
            nc.vector.tensor_scalar(m2[:], m2[:], -0.125, None, op0=ALU.mult)
            nc.vector.tensor_tensor(m2[:], m2[:], sQ[:], op=ALU.add)
            nc.vector.tensor_scalar(m2[:], m2[:], 1.0 / 7.0, 1e-30,
                                    op0=ALU.mult, op1=ALU.max)
            rv = _rsqrt(nc, tails, m2[:], 96, 96, tag="v", n_iter=1)
            st["sU"], st["m2"], st["rv"] = sU, m2, rv

        def pA4(d, st):
            # X = [tanh-in maps: scores | mean | std] then Pade(7,6) on Pool
            scsb = d["scsb"]
            sU, m2, rv = st["sU"], st["m2"], st["rv"]
            X = tails.tile([96, 288], f32, tag="X")
            nc.vector.tensor_copy(X[:, 0:96], scsb[:])
            nc.vector.tensor_scalar(X[:, 96:192], sU[:], 0.125, None, op0=ALU.mult)
            nc.vector.tensor_tensor(X[:, 192:288], m2[:], rv[:], op=ALU.mult)
            st["X"] = X

        def pA5(d, st):
            X = st["X"]
            t = tails.tile([96, 288], f32, tag="th_t")
            n = tails.tile([96, 288], f32, tag="th_n")
            nc.vector.tensor_tensor(t[:], X[:], X[:], op=ALU.mult)
            nc.vector.tensor_scalar(n[:], t[:], 378.0, None, op0=ALU.add)
            nc.vector.tensor_tensor(n[:], n[:], t[:], op=ALU.mult)
            nc.vector.tensor_scalar(n[:], n[:], 17325.0, None, op0=ALU.add)
            nc.vector.tensor_tensor(n[:], n[:], t[:], op=ALU.mult)
            nc.vector.tensor_scalar(n[:], n[:], 135135.0, None, op0=ALU.add)
            nc.vector.tensor_tensor(n[:], n[:], X[:], op=ALU.mult)
            st["th_t"], st["th_n"] = t, n

        def pA6(d, st):
            t, n = st["th_t"], st["th_n"]
            dn = tails.tile([96, 288], f32, tag="th_d")
            nc.vector.tensor_scalar(dn[:], t[:], 28.0, 3150.0, op0=ALU.mult, op1=ALU.add)
            nc.vector.tensor_tensor(dn[:], dn[:], t[:], op=ALU.mult)
            nc.vector.tensor_scalar(dn[:], dn[:], 62370.0, None, op0=ALU.add)
            nc.vector.tensor_tensor(dn[:], dn[:], t[:], op=ALU.mult)
            nc.vector.tensor_scalar(dn[:], dn[:], 135135.0, None, op0=ALU.add)
            nc.vector.reciprocal(dn[:], dn[:])
            thv = tails.tile([96, 288], f32r, tag="thv")
            nc.vector.tensor_tensor(thv[:], n[:], dn[:], op=ALU.mult)
            nc.vector.tensor_scalar(thv[:], thv[:], -1.0, 1.0, op0=ALU.max, op1=ALU.min)
            st["thv"] = thv

        def pF(d, st):
            s = d["s"]
            thv = st["thv"]
            brhs = brhsP[st["slot"] % 2]
            st["brhs"] = brhs
            for j in range(3):
                nc.sync.dma_start(
                    th_scr[s, j].rearrange("(a b) -> a b", a=96),
                    thv[:, j * 96:(j + 1) * 96])
            for j in range(3):
                for g in range(2):
                    nc.sync.dma_start(
                        brhs[:].rearrange("(sr gj) q -> gj sr q", gj=8)[4 * g + j],
                        th_scr[s, j, g * PIXH:(g + 1) * PIXH]
                        .rearrange("(sr q) -> sr q", q=CHW))
            c2sb = c2p.tile([128, 72], f32, tag="c2sb")
            st["c2sb"] = c2sb
            st["enc"] = {}
            st["r1"] = {}
            st["c2src"] = {}

        def chunk(d, st, k):
            brhs, c2sb = st["brhs"], st["c2sb"]
            if k < NCH:
                D = tbp.tile([128, 512], f32, tag="tb")
                nc.tensor.matmul(D[:, 0:CHW], blsb[k][:], brhs[:],
                                 start=True, stop=True)
                e1 = chks.tile([128, CHW], f32, tag="e1")
                nc.scalar.activation(e1[:], D[:, 0:CHW], AF.Abs, bias=zsb[:, 0:1])
                enc = chks.tile([128, CHW], f32r, tag="enc")
                nc.vector.tensor_scalar(enc[:], e1[:], -1.0, -1.0,
                                        op0=ALU.mult, op1=ALU.max)
                st["enc"][k] = enc
            if 0 <= k - 1 < NCH or 0 <= k - 2 < NCH:
                c1 = tbp.tile([128, 512], f32, tag="tb")
                kk = k - 2
                if 0 <= kk < NCH:
                    # conv2 for chunk k-2 rides in spare cols of this psum tile
                    for nn in range(3):
                        nc.tensor.matmul(c1[:, CHW + 2 * nn:CHW + 2 * nn + 2],
                                         st["r1"][kk][:, nn * 128:(nn + 1) * 128],
                                         w2sb[:], start=True, stop=True)
                    st["c2src"][kk] = c1
                if 0 <= k - 1 < NCH:
                    nc.tensor.matmul(c1[:, 0:CHW], w1sb[:], st["enc"].pop(k - 1)[:],
                                     start=True, stop=True)
                    r1 = chks.tile([128, CHW], f32r, tag="r1")
                    nc.vector.tensor_scalar(r1[:], c1[:, 0:CHW], b1sb[:, 0:1], 0.0,
                                            op0=ALU.add, op1=ALU.max)
                    st["r1"][k - 1] = r1
            kk = k - 2
            if 0 <= kk < NCH and kk in st["c2src"]:
                src = st["c2src"].pop(kk)
                st["r1"].pop(kk)
                nc.vector.tensor_copy(
                    c2sb[:].rearrange("p (g cc) -> p g cc", g=2)[:, :, 3 * kk:3 * kk + 3],
                    src[:, CHW:CHW + 6].rearrange("p (n g) -> p g n", g=2))

        def fin(d, st):
            s = d["s"]
            nc.sync.dma_start(
                outd[s].rearrange("g (c p) -> p g c", p=128),
                st["c2sb"][:].rearrange("p (g c) -> p g c", g=2))

        def tail_pieces(d, st):
            pa = [lambda: pA1(d, st), lambda: pA2(d, st), lambda: pA3(d, st),
                  lambda: pA4(d, st), lambda: pA5(d, st), lambda: pA6(d, st),
                  lambda: pF(d, st)]
            pb = [(lambda k: (lambda: chunk(d, st, k)))(k) for k in range(NCH + 2)]
            pb.append(lambda: fin(d, st))
            return pa, pb

        QA_SLOTS = {12, 15, 18, 21, 24, 27, 30}

        def weaver(qa, qb):
            def w(t):
                if t in QA_SLOTS and qa:
                    qa.pop(0)()
                elif qb and ((t % 2 == 1 and t >= 5 and t not in QA_SLOTS)
                             or t >= 31):
                    qb.pop(0)()
            return w

        # ---- main pipelined flow ----
        # tailA of seq i weaves into sim(i+1); its conv chunks into sim(i+2)
        NS = 2 * REPEAT
        qa, qb, qb_next = [], [], []
        H = {0: load(0)}
        for i in range(NS):
            if i + 1 < NS:
                H[i + 1] = load((i + 1) % SL)
            d = H.pop(i)
            sim(d, weave=weaver(qa, qb))
            for p in qa + qb:
                p()
            if STAGE >= 1:
                st = {"slot": i}
                pmtevac(d, st)
                qa, newb = tail_pieces(d, st)
                qb = qb_next
                qb_next = newb if STAGE >= 2 else []
            else:
                qa, qb, qb_next = [], [], []
        for p in qa + qb + qb_next:
            p()

    nc.compile()
    return nc


_prog = None


def kernel(**inputs) -> np.ndarray:
    global _prog
    test_scores = np.asarray(inputs["test_scores"], np.float32)
    train_labels = np.asarray(inputs["train_labels"], np.float32)
    test_feat = np.asarray(inputs["test_feat"], np.float32)
    train_feats = np.asarray(inputs["train_feats"], np.float32)
    temp = float(np.asarray(inputs["softmax_temp"]).reshape(-1)[0])
    conv1_w = np.asarray(inputs["conv1_w"], np.float32)
    conv1_b = np.asarray(inputs["conv1_b"], np.float32)
    bn_gamma = np.asarray(inputs["bn_gamma"], np.float32)
    bn_beta = np.asarray(inputs["bn_beta"], np.float32)
    bn_mean = np.asarray(inputs["bn_mean"], np.float32)
    bn_var = np.asarray(inputs["bn_var"], np.float32)
    conv2_w = np.asarray(inputs["conv2_w"], np.float32)
    conv2_b = np.asarray(inputs["conv2_b"], np.float32)

    R = resize_matrix(96, 24)
    labd = np.einsum("ik,mskl,jl->msij", R, train_labels, R)  # (M, NSEQ, 24, 24)
    lm_all = np.zeros((NSEQ, KTOT, 40), np.float32)
    for m in range(M):
        lm_all[:, m * L:(m + 1) * L, m] = labd[m].reshape(NSEQ, L)
        lm_all[:, m * L:(m + 1) * L, 32 + m] = 1.0
    lm_dev = lm_all.reshape(NSEQ, NKT, 128, 40).transpose(0, 2, 1, 3) \
        .reshape(NSEQ, 128, NKT * 40)

    s_o = np.sqrt(bn_var + BN_EPS)
    w1f = conv1_w * (bn_gamma / s_o)[:, None]
    b1f = (conv1_b - bn_mean) / s_o * bn_gamma + bn_beta
    b1f = b1f + w1f.sum(axis=1)   # kernel feeds enc-1; fold +1*W1 into bias
    W1 = np.zeros((128, 128), np.float32)
    W1[0:64, 0:64] = w1f.T
    W1[64:128, 64:128] = w1f.T
    W2 = np.zeros((128, 2), np.float32)
    W2[0:64, 0] = conv2_w[0]
    W2[64:128, 1] = conv2_w[0]
    B1 = np.concatenate([b1f, b1f]).reshape(128, 1)

    BL = np.zeros((8, 128), np.float32)
    for g in range(2):
        for ch in range(64):
            p = ch + 64 * g
            if ch < 32:
                j, a, b, c = 0, 15.5, 15.5, float(ch)
            elif ch < 48:
                j, a, b, c = 1, 15.0, 0.0, float(ch - 32)
            else:
                j, a, b, c = 2, 15.0, 0.0, float(ch - 48)
            BL[4 * g + j, p] = a
            BL[4 * g + 3, p] += b - c
    BL12 = np.zeros((12, 96, 128), np.float32)
    for bb in range(12):
        BL12[bb, 8 * bb:8 * bb + 8] = BL
    BRI = np.zeros((96, CHW), np.float32)
    for sr in range(NCH):
        for g in range(2):
            BRI[sr * 8 + 4 * g + 3, :] = 1.0

    UT = np.ascontiguousarray(resize_matrix(24, 96).T)

    tf_r = train_feats.reshape(M, NSEQ, C, L)
    te_r = test_feat.reshape(NSEQ, C, L)
    # host-computed norms: tes pre-normalized, per-key exp scale = temp/||tr_k||
    tes_all = te_r / np.sqrt((te_r * te_r).sum(axis=1, keepdims=True))
    nk = np.sqrt((tf_r * tf_r).sum(axis=2))          # (M, NSEQ, L)
    nkk = nk.transpose(1, 0, 2).reshape(NSEQ, KTOT)  # key order k = m*L + l
    sclk_all = (temp / nkk).reshape(NSEQ, NKT, 128).transpose(0, 2, 1)

    in_maps = []
    for c in range(NCORES):
        sl = slice(SL * c, SL * (c + 1))
        trc = np.ascontiguousarray(
            tf_r[:, sl].transpose(1, 2, 0, 3).reshape(SL, 2, 128, KTOT))
        tec = tes_all[sl].reshape(SL, 2, 128, L)
        pk = np.ascontiguousarray(np.concatenate(
            [tec[:, 0], tec[:, 1], sclk_all[sl].astype(np.float32),
             lm_dev[sl]], axis=2))
        tscc = np.ascontiguousarray(np.transpose(test_scores[0, sl], (0, 2, 1)))
        in_maps.append({
            "trf": trc, "pkin": pk,
            "tsc": tscc,
            "w1": W1, "w2": W2, "b1": B1, "binlhs": BL12, "utm": UT,
            "brini": BRI,
        })

    if _prog is None:
        _prog = build_program()
    res = run_bass_kernel_spmd(_prog, in_maps, core_ids=list(range(NCORES)))

    out = np.empty((1, NSEQ, WL, HL), np.float32)
    for c in range(NCORES):
        o = res.results[c]["out"]
        for s in range(SL):
            img_t = np.concatenate([o[s, 0], o[s, 1]]).reshape(96, 96)
            out[0, SL * c + s] = img_t.T + conv2_b[0]
    return out


if __name__ == "__main__":
    rng = np.random.default_rng(0)
    ins = {
        "test_scores": rng.standard_normal((1, NSEQ, WL, HL)).astype(np.float32),
        "train_labels": rng.uniform(0, 1, (M, NSEQ, WL, HL)).astype(np.float32),
        "test_feat": rng.standard_normal((1, NSEQ, C, WF, HF)).astype(np.float32),
        "train_feats": rng.standard_normal((M, NSEQ, C, WF, HF)).astype(np.float32),
        "softmax_temp": np.full((1,), 50.0, np.float32),
        "conv1_w": (rng.standard_normal((64, 64)) * 0.05).astype(np.float32),
        "conv1_b": np.zeros((64,), np.float32),
        "bn_gamma": np.ones((64,), np.float32),
        "bn_beta": np.zeros((64,), np.float32),
        "bn_mean": np.zeros((64,), np.float32),
        "bn_var": np.ones((64,), np.float32),
        "conv2_w": (rng.standard_normal((1, 64)) * 0.05).astype(np.float32),
        "conv2_b": np.zeros((1,), np.float32),
    }
    out = kernel(**ins)
    print("out shape:", out.shape, "mean", float(out.mean()), "std", float(out.std()))
